# revision 8
# baseline (speedup 1.0000x reference)
"""Trainium2 Bass kernel for the AlphaFold-style structure module.

Self-contained: hardcodes shapes/sharding. kernel(**inputs) -> tuple of outputs
matching the reference (frames7, sc44, unnorm, angles, pos, s).
"""
import numpy as np
from contextlib import ExitStack

import concourse.bass as bass
import concourse.mybir as mybir
import concourse.tile as tile
from concourse import bacc
from concourse.bass_utils import run_bass_kernel_spmd
from concourse.masks import make_identity

f32 = mybir.dt.float32
f32r = mybir.dt.float32r
AF = mybir.ActivationFunctionType
OP = mybir.AluOpType

B, N, CS, CH = 128, 512, 384, 128
NCORE = 8
MC = B // NCORE            # members per core (16)
TC = MC * N                # tokens per core (8192)
NFR, NAT, NRES = 8, 14, 21
EPS = 1e-8
TRANS_SCALE = 10.0
SBT = 16                   # token tiles per superblock


def _r(t):
    return t


def _cv(t, off, dims):
    """Free-dim view of a 3D tile [128, gg, C]: keeps partition+group dims,
    replaces comp dim with custom (step, count) dims at element offset off."""
    return bass.AP(tensor=t.tensor, offset=t.offset + off,
                   ap=[list(t.ap[0]), list(t.ap[1])] + [list(d) for d in dims])


def build_nc(caps, has_bias, use_f32r=True):
    """Emit the full Tile program for one core. caps: list of 21 per-class
    slot counts (multiples of 128)."""
    MDT = f32r if use_f32r else f32
    TDEV = int(sum(caps))
    assert TDEV % 128 == 0
    NT = TDEV // 128
    offs = np.concatenate([[0], np.cumsum(caps)]).astype(int)

    nc = bacc.Bacc("TRN2", target_bir_lowering=False, debug=False,
                   num_devices=NCORE)

    # ---- DRAM tensors ----
    sT_d = nc.dram_tensor("sT", [CS, TDEV], MDT, kind="ExternalInput").ap()
    siT_d = nc.dram_tensor("siT", [CS, TDEV], MDT, kind="ExternalInput").ap()
    rot_d = nc.dram_tensor("rot9", [TDEV, 9], f32, kind="ExternalInput").ap()
    tr_d = nc.dram_tensor("tr3", [TDEV, 3], f32, kind="ExternalInput").ap()
    oneh_d = nc.dram_tensor("oneh", [NRES, TDEV], f32, kind="ExternalInput").ap()
    dfl_d = nc.dram_tensor("dfl", [NRES, 96], f32, kind="ExternalInput").ap()
    pm_d = nc.dram_tensor("pmw", [NRES, 128, 42], f32, kind="ExternalInput").ap()
    wi_d = nc.dram_tensor("wi3", [3, CH, CH], MDT, kind="ExternalInput").ap()
    wn_d = nc.dram_tensor("wn3", [3, CH, CH], MDT, kind="ExternalInput").ap()
    b1w_d = nc.dram_tensor("bw1", [2, CH, CH], MDT, kind="ExternalInput").ap()
    b2w_d = nc.dram_tensor("bw2", [2, CH, CH], MDT, kind="ExternalInput").ap()
    wo_d = nc.dram_tensor("wo", [CH, NAT], MDT, kind="ExternalInput").ap()
    if has_bias:
        bc_d = nc.dram_tensor("bcomb", [CH], f32, kind="ExternalInput").ap()
        bb1_d = nc.dram_tensor("bb1", [2, CH], f32, kind="ExternalInput").ap()
        bb2_d = nc.dram_tensor("bb2", [2, CH], f32, kind="ExternalInput").ap()
        bo_d = nc.dram_tensor("bo", [NAT], f32, kind="ExternalInput").ap()

    sc_o = nc.dram_tensor("sc44_o", [TDEV, 128], f32, kind="ExternalOutput").ap()
    u_o = nc.dram_tensor("u_o", [TDEV, NAT], f32, kind="ExternalOutput").ap()
    ang_o = nc.dram_tensor("ang_o", [TDEV, NAT], f32, kind="ExternalOutput").ap()
    pos_o = nc.dram_tensor("pos_o", [TDEV, 42], f32, kind="ExternalOutput").ap()

    with tile.TileContext(nc) as tc, ExitStack() as ctx:
        wp = ctx.enter_context(tc.tile_pool(name="wp", bufs=1))
        stp = ctx.enter_context(tc.tile_pool(name="stp", bufs=3))
        rp = ctx.enter_context(tc.tile_pool(name="rp", bufs=2))
        big = ctx.enter_context(tc.tile_pool(name="big", bufs=2))
        tmp = ctx.enter_context(tc.tile_pool(name="tmp", bufs=3))
        psA = ctx.enter_context(tc.tile_pool(name="psA", bufs=3, space="PSUM"))
        psU = ctx.enter_context(tc.tile_pool(name="psU", bufs=2, space="PSUM"))
        psT = ctx.enter_context(tc.tile_pool(name="psT", bufs=3, space="PSUM"))

        # ---- preload constants ----
        ident = wp.tile([128, 128], f32)
        make_identity(nc, ident)
        wi_sb = wp.tile([CH, 3, CH], MDT, tag="wi")
        nc.sync.dma_start(wi_sb, wi_d.rearrange("c p m -> p c m"))
        wn_sb = wp.tile([CH, 3, CH], MDT, tag="wn")
        nc.sync.dma_start(wn_sb, wn_d.rearrange("c p m -> p c m"))
        b1_sb = wp.tile([CH, 2, CH], MDT, tag="b1w")
        nc.sync.dma_start(b1_sb, b1w_d.rearrange("c p m -> p c m"))
        b2_sb = wp.tile([CH, 2, CH], MDT, tag="b2w")
        nc.sync.dma_start(b2_sb, b2w_d.rearrange("c p m -> p c m"))
        wo_sb = wp.tile([CH, NAT], MDT, tag="wo")
        nc.sync.dma_start(wo_sb, wo_d)
        dfl_sb = wp.tile([NRES, 96], f32, tag="dfl")
        nc.sync.dma_start(dfl_sb, dfl_d)
        pm_sb = wp.tile([128, NRES, 42], f32, tag="pm")
        nc.sync.dma_start(pm_sb, pm_d.rearrange("r k m -> k r m"))
        if has_bias:
            bc_sb = wp.tile([CH, 1], f32, tag="bc")
            nc.sync.dma_start(bc_sb, bc_d[:, None])
            bb1_sb = wp.tile([CH, 2], f32, tag="bb1")
            nc.sync.dma_start(bb1_sb, bb1_d.rearrange("i c -> c i"))
            bb2_sb = wp.tile([CH, 2], f32, tag="bb2")
            nc.sync.dma_start(bb2_sb, bb2_d.rearrange("i c -> c i"))
            bo_sb = wp.tile([NAT, 1], f32, tag="bo")
            nc.sync.dma_start(bo_sb, bo_d[:, None])

        sT3 = sT_d.rearrange("(c p) t -> p c t", c=3)
        siT3 = siT_d.rearrange("(c p) t -> p c t", c=3)

        # global class runs [(r, lo, hi)) in device token space
        runs = [(r, int(offs[r]), int(offs[r + 1])) for r in range(NRES)
                if offs[r + 1] > offs[r]]

        def emit_front(t0, gg):
            TS = gg * 128
            utok = big.tile([128, gg, NAT], f32, tag="utok")
            drtok = big.tile([128, gg, 96], f32, tag="drtok")

            # ======== resnet + gathers (channel-major chunks of <=512) ========
            nbo = 0
            while nbo < TS:
                w = min(512, TS - nbo)
                c0 = t0 + nbo
                nq = w // 128
                sl = slice(c0, c0 + w)
                st_t = stp.tile([128, 3, w], MDT, tag="st")
                nc.sync.dma_start(st_t, sT3[:, :, sl])
                si_t = stp.tile([128, 3, w], MDT, tag="si")
                nc.sync.dma_start(si_t, siT3[:, :, sl])

                ps_x = psA.tile([128, w], f32, tag="mm")
                for c in range(3):
                    nc.tensor.matmul(ps_x, _r(wi_sb[:, c, :]), _r(si_t[:, c, :]),
                                     start=(c == 0), stop=False)
                for c in range(3):
                    nc.tensor.matmul(ps_x, _r(wn_sb[:, c, :]), _r(st_t[:, c, :]),
                                     start=False, stop=(c == 2))
                x_t = rp.tile([128, w], f32, tag="x")
                if has_bias:
                    nc.vector.tensor_scalar_add(x_t, ps_x, bc_sb[:, 0:1])
                    r_t = rp.tile([128, w], MDT, tag="r")
                    nc.scalar.activation(r_t, ps_x, AF.Relu, bias=bc_sb[:, 0:1])
                else:
                    nc.scalar.copy(x_t, ps_x)
                    r_t = rp.tile([128, w], MDT, tag="r")
                    nc.scalar.activation(r_t, ps_x, AF.Relu)
                for i in range(2):
                    ps_h = psA.tile([128, w], f32, tag="mm")
                    nc.tensor.matmul(ps_h, _r(b1_sb[:, i, :]), _r(r_t))
                    r2_t = rp.tile([128, w], MDT, tag="r2")
                    if has_bias:
                        nc.scalar.activation(r2_t, ps_h, AF.Relu,
                                             bias=bb1_sb[:, i:i + 1])
                    else:
                        nc.scalar.activation(r2_t, ps_h, AF.Relu)
                    ps_h2 = psA.tile([128, w], f32, tag="mm")
                    nc.tensor.matmul(ps_h2, _r(b2_sb[:, i, :]), _r(r2_t))
                    x2_t = rp.tile([128, w], f32, tag="x")
                    if has_bias:
                        nc.vector.scalar_tensor_tensor(
                            x2_t, ps_h2, bb2_sb[:, i:i + 1], x_t,
                            op0=OP.add, op1=OP.add)
                    else:
                        nc.vector.tensor_tensor(x2_t, ps_h2, x_t, op=OP.add)
                    r_t = rp.tile([128, w], MDT, tag="r")
                    nc.vector.tensor_scalar_max(r_t, x2_t, 0.0)
                    x_t = x2_t
                ps_u = psU.tile([NAT, w], f32, tag="u")
                nc.tensor.matmul(ps_u, _r(wo_sb), _r(r_t))
                ucm_t = rp.tile([NAT, w], f32, tag="ucm")
                if has_bias:
                    nc.vector.tensor_scalar_add(ucm_t, ps_u, bo_sb[:, 0:1])
                else:
                    nc.scalar.copy(ucm_t, ps_u)
                # transpose u to token-major
                ps_ut = psT.tile([128, 4 * NAT], f32, tag="tp")
                for q in range(nq):
                    nc.tensor.transpose(ps_ut[:, q * NAT:(q + 1) * NAT],
                                        ucm_t[:, q * 128:(q + 1) * 128],
                                        ident[0:NAT, 0:NAT])
                qb = nbo // 128
                nc.scalar.copy(
                    utok[:, qb:qb + nq, :],
                    bass.AP(tensor=ps_ut.tensor, offset=ps_ut.offset,
                            ap=[list(ps_ut.ap[0]), [NAT, nq], [1, NAT]]))

                # gathers: dfl[aatype] via one-hot matmul
                oneh_t = stp.tile([NRES, w], f32, tag="oneh")
                nc.sync.dma_start(oneh_t, oneh_d[:, sl])
                ps_d = psT.tile([96, w], f32, tag="tp")
                nc.tensor.matmul(ps_d, _r(dfl_sb), _r(oneh_t))
                dcm_t = rp.tile([96, w], f32, tag="dcm")
                nc.scalar.copy(dcm_t, ps_d)
                ps_dt = psT.tile([128, 4 * 96, ], f32, tag="tp")
                for q in range(nq):
                    nc.tensor.transpose(ps_dt[:, q * 96:(q + 1) * 96],
                                        dcm_t[:, q * 128:(q + 1) * 128],
                                        ident[0:96, 0:96])
                nc.vector.tensor_copy(
                    drtok[:, qb:qb + nq, :],
                    bass.AP(tensor=ps_dt.tensor, offset=ps_dt.offset,
                            ap=[list(ps_dt.ap[0]), [96, nq], [1, 96]]))
                nbo += w
            return utok, drtok

        def emit_back(t0, gg, utok, drtok):
            TS = gg * 128
            rot_t = big.tile([128, gg, 9], f32, tag="rot")
            nc.sync.dma_start(rot_t,
                              rot_d[t0:t0 + TS].rearrange("(g p) c -> p g c", p=128))
            tr_t = big.tile([128, gg, 3], f32, tag="tr")
            nc.sync.dma_start(tr_t,
                              tr_d[t0:t0 + TS].rearrange("(g p) c -> p g c", p=128))

            # --- angles ---
            sq_t = tmp.tile([128, gg, NAT], f32, tag="t224")
            nc.vector.tensor_tensor(sq_t, utok, utok, op=OP.mult)
            ps7 = tmp.tile([128, gg, 7], f32, tag="t112a")
            nc.vector.tensor_tensor(ps7, sq_t[:, :, 0:NAT:2], sq_t[:, :, 1:NAT:2],
                                    op=OP.add)
            nc.vector.tensor_scalar_max(ps7, ps7, EPS)
            sr7 = tmp.tile([128, gg, 7], f32, tag="t112b")
            nc.scalar.activation(sr7, ps7, AF.Sqrt)
            rn7 = tmp.tile([128, gg, 7], f32, tag="t112c")
            nc.vector.reciprocal(rn7, sr7)
            sincos = big.tile([128, gg, 16], f32, tag="sincos")
            nc.gpsimd.memset(sincos[:, :, 0:1], 0.0)
            nc.gpsimd.memset(sincos[:, :, 1:2], 1.0)
            nc.vector.tensor_tensor(_cv(sincos, 2, [(2, 7), (1, 2)]),
                                    _cv(utok, 0, [(2, 7), (1, 2)]),
                                    _cv(rn7, 0, [(1, 7), (0, 2)]), op=OP.mult)
            nc.sync.dma_start(u_o[t0:t0 + TS].rearrange("(g p) c -> p g c", p=128),
                              utok)
            nc.sync.dma_start(ang_o[t0:t0 + TS].rearrange("(g p) c -> p g c", p=128),
                              sincos[:, :, 2:16])

            # --- rotx compose: fR = dR @ rotx(angle) ---
            Rbb = big.tile([128, gg, 72], f32, tag="Rbb")
            fRt = big.tile([128, gg, 27], f32, tag="fRt")
            tbb = big.tile([128, gg, 24], f32, tag="tbb")

            def rotx(dst, doff, fs, cnt, eng):
                dv = lambda j, st=9: _cv(dst, doff + j, [(st, cnt), (3, 3)])
                sv = lambda j: _cv(drtok, fs * 9 + j, [(9, cnt), (3, 3)])
                cb = _cv(sincos, fs * 2 + 1, [(2, cnt), (0, 3)])
                sb_ = _cv(sincos, fs * 2 + 0, [(2, cnt), (0, 3)])
                eng.tensor_copy(dv(0), sv(0))
                ta = tmp.tile([128, gg, cnt * 3], f32, tag="t240a")
                tav = _cv(ta, 0, [(3, cnt), (1, 3)])
                tb = tmp.tile([128, gg, cnt * 3], f32, tag="t240b")
                tbv = _cv(tb, 0, [(3, cnt), (1, 3)])
                eng.tensor_tensor(tav, sv(1), cb, op=OP.mult)
                eng.tensor_tensor(tbv, sv(2), sb_, op=OP.mult)
                eng.tensor_tensor(dv(1), tav, tbv, op=OP.add)
                eng.tensor_tensor(tav, sv(2), cb, op=OP.mult)
                eng.tensor_tensor(tbv, sv(1), sb_, op=OP.mult)
                eng.tensor_tensor(dv(2), tav, tbv, op=OP.subtract)

            rotx(Rbb, 0, 0, 5, nc.vector)
            rotx(fRt, 0, 5, 3, nc.gpsimd)
            nc.gpsimd.tensor_copy(_cv(tbb, 0, [(1, 15)]), _cv(drtok, 72, [(1, 15)]))

            # --- chain composes: frames 5,6,7 ---
            for kidx, f in enumerate([5, 6, 7]):
                af = 4 + kidx          # A frame slot in Rbb/tbb
                df = 5 + kidx          # dest slot
                soff = (f - 5) * 9     # source in fRt
                DV = _cv(Rbb, df * 9, [(3, 3), (1, 3)])
                t9 = tmp.tile([128, gg, 9], f32, tag="t144")
                t9v = _cv(t9, 0, [(3, 3), (1, 3)])
                for k in range(3):
                    Ak = _cv(Rbb, af * 9 + k, [(3, 3), (0, 3)])
                    Bk = _cv(fRt, soff + k * 3, [(0, 3), (1, 3)])
                    if k == 0:
                        nc.vector.tensor_tensor(DV, Ak, Bk, op=OP.mult)
                    else:
                        nc.vector.tensor_tensor(t9v, Ak, Bk, op=OP.mult)
                        nc.vector.tensor_tensor(DV, DV, t9v, op=OP.add)
                tv = _cv(tbb, df * 3, [(1, 3)])
                t3 = tmp.tile([128, gg, 3], f32, tag="t48")
                t3v = _cv(t3, 0, [(1, 3)])
                for k in range(3):
                    Ak = _cv(Rbb, af * 9 + k, [(3, 3)])
                    Btk = _cv(drtok, 72 + f * 3 + k, [(0, 3)])
                    if k == 0:
                        nc.gpsimd.tensor_tensor(tv, Ak, Btk, op=OP.mult)
                    else:
                        nc.gpsimd.tensor_tensor(t3v, Ak, Btk, op=OP.mult)
                        nc.gpsimd.tensor_tensor(tv, tv, t3v, op=OP.add)
                nc.gpsimd.tensor_tensor(tv, tv, _cv(tbb, af * 3, [(1, 3)]),
                                        op=OP.add)

            # --- backbone compose into sc44 layout ---
            sc44 = big.tile([128, gg, 128], f32, tag="sc44")
            nc.gpsimd.memset(_cv(sc44, 12, [(16, 8), (1, 3)]), 0.0)
            nc.gpsimd.memset(_cv(sc44, 15, [(16, 8)]), 1.0)
            t24 = tmp.tile([128, gg, 24], f32, tag="t24")
            t24v = _cv(t24, 0, [(3, 8), (1, 3)])
            for i in range(3):
                for k in range(3):
                    in0 = _cv(Rbb, k * 3, [(9, 8), (1, 3)])
                    in1 = _cv(rot_t, i * 3 + k, [(0, 8), (0, 3)])
                    dst = _cv(sc44, i * 4, [(16, 8), (1, 3)])
                    if k == 0:
                        nc.vector.tensor_tensor(dst, in0, in1, op=OP.mult)
                    else:
                        nc.vector.tensor_tensor(t24v, in0, in1, op=OP.mult)
                        nc.vector.tensor_tensor(dst, dst, t24v, op=OP.add)
            t8 = tmp.tile([128, gg, 8], f32, tag="t8")
            t8v = _cv(t8, 0, [(1, 8)])
            for i in range(3):
                dst = _cv(sc44, i * 4 + 3, [(16, 8)])
                for k in range(3):
                    in0 = _cv(tbb, k, [(3, 8)])
                    in1 = _cv(rot_t, i * 3 + k, [(0, 8)])
                    if k == 0:
                        nc.gpsimd.tensor_tensor(dst, in0, in1, op=OP.mult)
                    else:
                        nc.gpsimd.tensor_tensor(t8v, in0, in1, op=OP.mult)
                        nc.gpsimd.tensor_tensor(dst, dst, t8v, op=OP.add)
                nc.gpsimd.tensor_tensor(dst, dst, _cv(tr_t, i, [(0, 8)]),
                                        op=OP.add)
            nc.sync.dma_start(sc_o[t0:t0 + TS].rearrange("(g p) c -> p g c", p=128),
                              sc44)

            # --- transpose sc44 back to channel-major for per-class pos matmuls
            rtc = big.tile([128, gg * 128], f32, tag="rtc")
            for qq in range((gg + 3) // 4):
                n4 = min(4, gg - qq * 4)
                ps_rt = psT.tile([128, 512], f32, tag="tp")
                for q2 in range(n4):
                    nc.tensor.transpose(ps_rt[:, q2 * 128:(q2 + 1) * 128],
                                        sc44[:, qq * 4 + q2, :], ident)
                nc.scalar.copy(rtc[:, qq * 512:qq * 512 + n4 * 128],
                               ps_rt[:, 0:n4 * 128])

            # --- per-class pos matmuls ---
            poscm = big.tile([42, gg * 128], f32, tag="poscm")
            nbo = 0
            while nbo < TS:
                w = min(512, TS - nbo)
                ps_pos = psT.tile([42, 512], f32, tag="tp")
                wlo, whi = t0 + nbo, t0 + nbo + w
                for (r, lo, hi) in runs:
                    a, b_ = max(lo, wlo), min(hi, whi)
                    if a >= b_:
                        continue
                    nc.tensor.matmul(ps_pos[:, a - wlo:b_ - wlo],
                                     _r(pm_sb[:, r, :]),
                                     _r(rtc[:, a - t0:b_ - t0]),
                                     start=True, stop=True)
                nc.vector.tensor_copy(poscm[:, nbo:nbo + w], ps_pos[:, 0:w])
                nbo += w

            # --- transpose pos to token-major & store ---
            postok = big.tile([128, gg, 42], f32, tag="postok")
            for qq in range((gg + 3) // 4):
                n4 = min(4, gg - qq * 4)
                ps_pt = psT.tile([128, 4 * 42], f32, tag="tp")
                for q2 in range(n4):
                    nc.tensor.transpose(ps_pt[:, q2 * 42:(q2 + 1) * 42],
                                        poscm[:, (qq * 4 + q2) * 128:(qq * 4 + q2 + 1) * 128],
                                        ident[0:42, 0:42])
                nc.vector.tensor_copy(
                    postok[:, qq * 4:qq * 4 + n4, :],
                    bass.AP(tensor=ps_pt.tensor, offset=ps_pt.offset,
                            ap=[list(ps_pt.ap[0]), [42, n4], [1, 42]]))
            nc.sync.dma_start(pos_o[t0:t0 + TS].rearrange("(g p) c -> p g c", p=128),
                              postok)

        # ---- software-pipelined emission: front(k+1) before back(k) so the
        # PE instruction stream never stalls long on DVE structure output ----
        sbs = []
        t0 = 0
        while t0 < TDEV:
            gg = min(SBT, NT - t0 // 128)
            sbs.append((t0, gg))
            t0 += gg * 128
        pend = None
        for (s0, sg) in sbs:
            ut, dr = emit_front(s0, sg)
            if pend is not None:
                emit_back(*pend)
            pend = (s0, sg, ut, dr)
        emit_back(*pend)

    nc.compile()
    return nc


# ---------------- host staging ----------------

def _tf32_round(a):
    """Round-to-nearest-even to f32r precision (8-bit exp, 11-bit mantissa)."""
    b = np.ascontiguousarray(a, np.float32).view(np.uint32)
    lsb = (b >> 12) & 1
    r = (b + 0x7FF + lsb) & np.uint32(0xFFFFF000)
    return r.view(np.float32)


def _stage(inputs, use_f32r=True):
    s = np.ascontiguousarray(inputs["s"], np.float32)
    si = np.ascontiguousarray(inputs["s_initial"], np.float32)
    rot = np.ascontiguousarray(inputs["rot_mats"], np.float32)
    tr = np.ascontiguousarray(inputs["trans"], np.float32)
    aat = np.asarray(inputs["aatype"]).astype(np.int32)
    df = np.ascontiguousarray(inputs["default_frames"], np.float32)
    gidx = np.asarray(inputs["group_idx"]).astype(np.int64)
    amask = np.ascontiguousarray(inputs["atom_mask"], np.float32)
    lit = np.ascontiguousarray(inputs["lit_positions"], np.float32)

    counts = np.zeros((NCORE, NRES), np.int64)
    aat_c = aat.reshape(NCORE, TC)
    for c in range(NCORE):
        counts[c] = np.bincount(aat_c[c], minlength=NRES)
    caps = (np.ceil(counts.max(0) / 128).astype(int) * 128).clip(128)
    TDEV = int(caps.sum())
    offs = np.concatenate([[0], np.cumsum(caps)]).astype(int)

    # tables
    dR = df[:, :, :3, :3].reshape(NRES, 72)
    dt = df[:, :, :3, 3].reshape(NRES, 24)
    dfl = np.ascontiguousarray(np.concatenate([dR, dt], 1), np.float32)
    pmw = np.zeros((NRES, 128, 42), np.float32)
    rr, aa_ = np.meshgrid(np.arange(NRES), np.arange(NAT), indexing="ij")
    g_of = gidx  # [21,14]
    for i in range(3):
        for jj in range(4):
            val = amask * (lit[:, :, jj] if jj < 3 else 1.0)
            pmw[rr, g_of * 16 + i * 4 + jj, aa_ * 3 + i] = val

    w_init = np.ascontiguousarray(inputs["w_init"], np.float32)
    w_in = np.ascontiguousarray(inputs["w_in"], np.float32)
    wi3 = np.ascontiguousarray(w_init.reshape(3, 128, CH))
    wn3 = np.ascontiguousarray(w_in.reshape(3, 128, CH))
    bw1 = np.ascontiguousarray(inputs["blk_w1"], np.float32)
    bw2 = np.ascontiguousarray(inputs["blk_w2"], np.float32)
    wo = np.ascontiguousarray(inputs["w_out"], np.float32)
    b_init = np.asarray(inputs["b_init"], np.float32)
    b_in = np.asarray(inputs["b_in"], np.float32)
    bb1 = np.ascontiguousarray(inputs["blk_b1"], np.float32)
    bb2 = np.ascontiguousarray(inputs["blk_b2"], np.float32)
    bo = np.asarray(inputs["b_out"], np.float32)
    has_bias = bool(np.any(b_init) or np.any(b_in) or np.any(bb1)
                    or np.any(bb2) or np.any(bo))

    if use_f32r:
        wi3, wn3, bw1, bw2, wo = (_tf32_round(x) for x in (wi3, wn3, bw1, bw2, wo))
    shared = dict(dfl=dfl, pmw=pmw, wi3=wi3, wn3=wn3, bw1=bw1, bw2=bw2, wo=wo)
    if has_bias:
        shared.update(bcomb=(b_init + b_in).astype(np.float32), bb1=bb1,
                      bb2=bb2, bo=bo)

    in_maps, slot_maps = [], []
    s2 = s.reshape(NCORE, TC, CS)
    si2 = si.reshape(NCORE, TC, CS)
    rot2 = rot.reshape(NCORE, TC, 9)
    tr2 = tr.reshape(NCORE, TC, 3)
    for c in range(NCORE):
        a = aat_c[c]
        order = np.argsort(a, kind="stable")
        cnt = counts[c]
        # slot index for each source token
        rank = np.empty(TC, np.int64)
        rank[order] = np.arange(TC)
        csum = np.concatenate([[0], np.cumsum(cnt)])
        slot_of_src = offs[a] + (rank - csum[a])
        sT = np.zeros((CS, TDEV), np.float32)
        sT[:, slot_of_src] = np.maximum(s2[c], 0).T
        siT = np.zeros((CS, TDEV), np.float32)
        siT[:, slot_of_src] = np.maximum(si2[c], 0).T
        if use_f32r:
            sT = _tf32_round(sT)
            siT = _tf32_round(siT)
        rot9 = np.zeros((TDEV, 9), np.float32)
        rot9[slot_of_src] = rot2[c]
        tr3 = np.zeros((TDEV, 3), np.float32)
        tr3[slot_of_src] = tr2[c] * TRANS_SCALE
        oneh = np.zeros((NRES, TDEV), np.float32)
        oneh[a, slot_of_src] = 1.0
        m = dict(shared)
        m.update(sT=sT, siT=siT, rot9=rot9, tr3=tr3, oneh=oneh)
        in_maps.append(m)
        slot_maps.append(slot_of_src)
    return in_maps, slot_maps, tuple(int(x) for x in caps), has_bias


def _frames7(rot_mats, trans):
    import jax
    import jax.numpy as jnp
    cpu = jax.devices("cpu")[0]
    with jax.default_device(cpu):
        rot = jnp.asarray(rot_mats)
        xx, xy, xz = rot[..., 0, 0], rot[..., 0, 1], rot[..., 0, 2]
        yx, yy, yz = rot[..., 1, 0], rot[..., 1, 1], rot[..., 1, 2]
        zx, zy, zz = rot[..., 2, 0], rot[..., 2, 1], rot[..., 2, 2]
        k = jnp.stack([
            jnp.stack([xx + yy + zz, zy - yz, xz - zx, yx - xy], -1),
            jnp.stack([zy - yz, xx - yy - zz, xy + yx, xz + zx], -1),
            jnp.stack([xz - zx, xy + yx, yy - xx - zz, yz + zy], -1),
            jnp.stack([yx - xy, xz + zx, yz + zy, zz - xx - yy], -1),
        ], -2) / 3.0
        _, vecs = jnp.linalg.eigh(k)
        quat = vecs[..., -1]
        bb_t = jnp.asarray(trans) * TRANS_SCALE
        out = jnp.concatenate([quat, bb_t], axis=-1)
        return np.asarray(out)


_CACHE = {}
TRACE = False
USE_F32R = False
LAST_RESULTS = None


def kernel(**inputs):
    global LAST_RESULTS
    in_maps, slot_maps, caps, has_bias = _stage(inputs, USE_F32R)
    key = (caps, has_bias, USE_F32R)
    if key not in _CACHE:
        _CACHE[key] = build_nc(list(caps), has_bias, USE_F32R)
    nc = _CACHE[key]
    rr = run_bass_kernel_spmd(nc, in_maps, core_ids=list(range(NCORE)),
                              trace=TRACE)
    LAST_RESULTS = rr
    res = rr.results

    sc44 = np.empty((B, N, 8, 4, 4), np.float32)
    unnorm = np.empty((B, N, 7, 2), np.float32)
    angles = np.empty((B, N, 7, 2), np.float32)
    pos = np.empty((B, N, NAT, 3), np.float32)
    for c in range(NCORE):
        sl = slot_maps[c]
        bs = slice(c * MC, (c + 1) * MC)
        sc44[bs] = res[c]["sc44_o"][sl].reshape(MC, N, 8, 4, 4)
        unnorm[bs] = res[c]["u_o"][sl].reshape(MC, N, 7, 2)
        angles[bs] = res[c]["ang_o"][sl].reshape(MC, N, 7, 2)
        pos[bs] = res[c]["pos_o"][sl].reshape(MC, N, NAT, 3)

    frames7 = _frames7(np.asarray(inputs["rot_mats"], np.float32),
                       np.asarray(inputs["trans"], np.float32))
    s_out = np.asarray(inputs["s"], np.float32)
    return frames7, sc44, unnorm, angles, pos, s_out


# revision 10
# speedup vs baseline: 1.5642x; 1.5642x over previous
"""Trainium2 Bass kernel for the AlphaFold-style structure module.

Self-contained: hardcodes shapes/sharding. kernel(**inputs) -> tuple of outputs
matching the reference (frames7, sc44, unnorm, angles, pos, s).
"""
import numpy as np
from contextlib import ExitStack

import concourse.bass as bass
import concourse.mybir as mybir
import concourse.tile as tile
from concourse import bacc
from concourse.bass_utils import run_bass_kernel_spmd
from concourse.masks import make_identity

f32 = mybir.dt.float32
f32r = mybir.dt.float32r
AF = mybir.ActivationFunctionType
OP = mybir.AluOpType

B, N, CS, CH = 128, 512, 384, 128
NCORE = 8
MC = B // NCORE            # members per core (16)
TC = MC * N                # tokens per core (8192)
NFR, NAT, NRES = 8, 14, 21
EPS = 1e-8
TRANS_SCALE = 10.0
SBT = 16                   # token tiles per superblock


def _r(t):
    return t


def _cv(t, off, dims):
    """Free-dim view of a 3D tile [128, gg, C]: keeps partition+group dims,
    replaces comp dim with custom (step, count) dims at element offset off."""
    return bass.AP(tensor=t.tensor, offset=t.offset + off,
                   ap=[list(t.ap[0]), list(t.ap[1])] + [list(d) for d in dims])


def build_nc(caps, has_bias, use_f32r=True):
    """Emit the full Tile program for one core. caps: list of 21 per-class
    slot counts (multiples of 128)."""
    MDT = f32r if use_f32r else f32
    TDEV = int(sum(caps))
    assert TDEV % 128 == 0
    NT = TDEV // 128
    offs = np.concatenate([[0], np.cumsum(caps)]).astype(int)

    nc = bacc.Bacc("TRN2", target_bir_lowering=False, debug=False,
                   num_devices=NCORE)

    # ---- DRAM tensors ----
    sT_d = nc.dram_tensor("sT", [CS, TDEV], MDT, kind="ExternalInput").ap()
    siT_d = nc.dram_tensor("siT", [CS, TDEV], MDT, kind="ExternalInput").ap()
    rot_d = nc.dram_tensor("rot9", [TDEV, 9], f32, kind="ExternalInput").ap()
    tr_d = nc.dram_tensor("tr3", [TDEV, 3], f32, kind="ExternalInput").ap()
    oneh_d = nc.dram_tensor("oneh", [NRES, TDEV], f32, kind="ExternalInput").ap()
    dfl_d = nc.dram_tensor("dfl", [NRES, 96], f32, kind="ExternalInput").ap()
    pm_d = nc.dram_tensor("pmw", [NRES, 128, 42], f32, kind="ExternalInput").ap()
    wi_d = nc.dram_tensor("wi3", [3, CH, CH], MDT, kind="ExternalInput").ap()
    wn_d = nc.dram_tensor("wn3", [3, CH, CH], MDT, kind="ExternalInput").ap()
    b1w_d = nc.dram_tensor("bw1", [2, CH, CH], MDT, kind="ExternalInput").ap()
    b2w_d = nc.dram_tensor("bw2", [2, CH, CH], MDT, kind="ExternalInput").ap()
    wo_d = nc.dram_tensor("wo", [CH, NAT], MDT, kind="ExternalInput").ap()
    if has_bias:
        bc_d = nc.dram_tensor("bcomb", [CH], f32, kind="ExternalInput").ap()
        bb1_d = nc.dram_tensor("bb1", [2, CH], f32, kind="ExternalInput").ap()
        bb2_d = nc.dram_tensor("bb2", [2, CH], f32, kind="ExternalInput").ap()
        bo_d = nc.dram_tensor("bo", [NAT], f32, kind="ExternalInput").ap()

    sc_o = nc.dram_tensor("sc44_o", [TDEV, 128], f32, kind="ExternalOutput").ap()
    u_o = nc.dram_tensor("u_o", [TDEV, NAT], f32, kind="ExternalOutput").ap()
    ang_o = nc.dram_tensor("ang_o", [TDEV, NAT], f32, kind="ExternalOutput").ap()
    pos_o = nc.dram_tensor("pos_o", [TDEV, 42], f32, kind="ExternalOutput").ap()

    with tile.TileContext(nc) as tc, ExitStack() as ctx:
        wp = ctx.enter_context(tc.tile_pool(name="wp", bufs=1))
        stp = ctx.enter_context(tc.tile_pool(name="stp", bufs=4))
        rp = ctx.enter_context(tc.tile_pool(name="rp", bufs=5))
        rp4 = ctx.enter_context(tc.tile_pool(name="rp4", bufs=4))
        big = ctx.enter_context(tc.tile_pool(name="big", bufs=2))
        tmp = ctx.enter_context(tc.tile_pool(name="tmp", bufs=2))
        psA = ctx.enter_context(tc.tile_pool(name="psA", bufs=4, space="PSUM"))
        psU = ctx.enter_context(tc.tile_pool(name="psU", bufs=1, space="PSUM"))
        psT = ctx.enter_context(tc.tile_pool(name="psT", bufs=2, space="PSUM"))

        # ---- preload constants ----
        ident = wp.tile([128, 128], f32)
        make_identity(nc, ident)
        wi_sb = wp.tile([CH, 3, CH], MDT, tag="wi")
        nc.sync.dma_start(wi_sb, wi_d.rearrange("c p m -> p c m"))
        wn_sb = wp.tile([CH, 3, CH], MDT, tag="wn")
        nc.sync.dma_start(wn_sb, wn_d.rearrange("c p m -> p c m"))
        b1_sb = wp.tile([CH, 2, CH], MDT, tag="b1w")
        nc.sync.dma_start(b1_sb, b1w_d.rearrange("c p m -> p c m"))
        b2_sb = wp.tile([CH, 2, CH], MDT, tag="b2w")
        nc.sync.dma_start(b2_sb, b2w_d.rearrange("c p m -> p c m"))
        wo_sb = wp.tile([CH, NAT], MDT, tag="wo")
        nc.sync.dma_start(wo_sb, wo_d)
        dfl_sb = wp.tile([NRES, 96], f32, tag="dfl")
        nc.sync.dma_start(dfl_sb, dfl_d)
        pm_sb = wp.tile([128, NRES, 42], f32, tag="pm")
        nc.sync.dma_start(pm_sb, pm_d.rearrange("r k m -> k r m"))
        if has_bias:
            bc_sb = wp.tile([CH, 1], f32, tag="bc")
            nc.sync.dma_start(bc_sb, bc_d[:, None])
            bb1_sb = wp.tile([CH, 2], f32, tag="bb1")
            nc.sync.dma_start(bb1_sb, bb1_d.rearrange("i c -> c i"))
            bb2_sb = wp.tile([CH, 2], f32, tag="bb2")
            nc.sync.dma_start(bb2_sb, bb2_d.rearrange("i c -> c i"))
            bo_sb = wp.tile([NAT, 1], f32, tag="bo")
            nc.sync.dma_start(bo_sb, bo_d[:, None])

        sT3 = sT_d.rearrange("(c p) t -> p c t", c=3)
        siT3 = siT_d.rearrange("(c p) t -> p c t", c=3)

        # global class runs [(r, lo, hi)) in device token space
        runs = [(r, int(offs[r]), int(offs[r + 1])) for r in range(NRES)
                if offs[r + 1] > offs[r]]

        def emit_front(t0, gg):
            TS = gg * 128
            utok = big.tile([128, gg, NAT], f32, tag="utok")
            drtok = big.tile([128, gg, 96], f32, tag="drtok")
            chunks = []
            nbo = 0
            while nbo < TS:
                w = min(512, TS - nbo)
                chunks.append((nbo, w))
                nbo += w
            # wave 0: DMAs
            st_l, si_l, on_l = [], [], []
            for (nbo, w) in chunks:
                sl = slice(t0 + nbo, t0 + nbo + w)
                st_t = stp.tile([128, 3, w], MDT, tag="st")
                nc.sync.dma_start(st_t, sT3[:, :, sl])
                si_t = stp.tile([128, 3, w], MDT, tag="si")
                nc.sync.dma_start(si_t, siT3[:, :, sl])
                oneh_t = stp.tile([NRES, w], f32, tag="oneh")
                nc.sync.dma_start(oneh_t, oneh_d[:, sl])
                st_l.append(st_t); si_l.append(si_t); on_l.append(oneh_t)
            # wave 1: L1 matmuls
            psx_l = []
            for ci, (nbo, w) in enumerate(chunks):
                ps_x = psA.tile([128, w], f32, tag="mm")
                for c in range(3):
                    nc.tensor.matmul(ps_x, _r(wi_sb[:, c, :]), _r(si_l[ci][:, c, :]),
                                     start=(c == 0), stop=False)
                for c in range(3):
                    nc.tensor.matmul(ps_x, _r(wn_sb[:, c, :]), _r(st_l[ci][:, c, :]),
                                     start=False, stop=(c == 2))
                psx_l.append(ps_x)
            # wave 2: x copy + relu
            x_l, r_l = [], []
            for ci, (nbo, w) in enumerate(chunks):
                x_t = rp.tile([128, w], f32, tag="x")
                r_t = rp.tile([128, w], MDT, tag="r")
                if has_bias:
                    nc.vector.tensor_scalar_add(x_t, psx_l[ci], bc_sb[:, 0:1])
                    nc.scalar.activation(r_t, psx_l[ci], AF.Relu, bias=bc_sb[:, 0:1])
                else:
                    nc.scalar.copy(x_t, psx_l[ci])
                    nc.scalar.activation(r_t, psx_l[ci], AF.Relu)
                x_l.append(x_t); r_l.append(r_t)
            # residual blocks, wave per layer
            for i in range(2):
                psh_l = []
                for ci, (nbo, w) in enumerate(chunks):
                    ps_h = psA.tile([128, w], f32, tag="mm")
                    nc.tensor.matmul(ps_h, _r(b1_sb[:, i, :]), _r(r_l[ci]))
                    psh_l.append(ps_h)
                r2_l = []
                for ci, (nbo, w) in enumerate(chunks):
                    r2_t = rp4.tile([128, w], MDT, tag="r2")
                    if has_bias:
                        nc.scalar.activation(r2_t, psh_l[ci], AF.Relu,
                                             bias=bb1_sb[:, i:i + 1])
                    else:
                        nc.scalar.activation(r2_t, psh_l[ci], AF.Relu)
                    r2_l.append(r2_t)
                psh2_l = []
                for ci, (nbo, w) in enumerate(chunks):
                    ps_h2 = psA.tile([128, w], f32, tag="mm")
                    nc.tensor.matmul(ps_h2, _r(b2_sb[:, i, :]), _r(r2_l[ci]))
                    psh2_l.append(ps_h2)
                for ci, (nbo, w) in enumerate(chunks):
                    x2_t = rp.tile([128, w], f32, tag="x")
                    if has_bias:
                        nc.vector.scalar_tensor_tensor(
                            x2_t, psh2_l[ci], bb2_sb[:, i:i + 1], x_l[ci],
                            op0=OP.add, op1=OP.add)
                    else:
                        nc.vector.tensor_tensor(x2_t, psh2_l[ci], x_l[ci],
                                                op=OP.add)
                    r_t = rp.tile([128, w], MDT, tag="r")
                    nc.vector.tensor_scalar_max(r_t, x2_t, 0.0)
                    x_l[ci] = x2_t; r_l[ci] = r_t
            # out layer + gathers
            psu_l, psd_l = [], []
            for ci, (nbo, w) in enumerate(chunks):
                ps_u = psU.tile([NAT, w], f32, tag="u")
                nc.tensor.matmul(ps_u, _r(wo_sb), _r(r_l[ci]))
                psu_l.append(ps_u)
                ps_d = psU.tile([96, w], f32, tag="d")
                nc.tensor.matmul(ps_d, dfl_sb, on_l[ci])
                psd_l.append(ps_d)
            ucm_l, dcm_l = [], []
            for ci, (nbo, w) in enumerate(chunks):
                ucm_t = rp4.tile([NAT, w], f32, tag="ucm")
                if has_bias:
                    nc.vector.tensor_scalar_add(ucm_t, psu_l[ci], bo_sb[:, 0:1])
                else:
                    nc.scalar.copy(ucm_t, psu_l[ci])
                ucm_l.append(ucm_t)
                dcm_t = rp4.tile([96, w], f32, tag="dcm")
                nc.scalar.copy(dcm_t, psd_l[ci])
                dcm_l.append(dcm_t)
            # transposes to token-major
            for ci, (nbo, w) in enumerate(chunks):
                nq = w // 128
                qb = nbo // 128
                ps_ut = psT.tile([128, 4 * NAT], f32, tag="tp")
                for q in range(nq):
                    nc.tensor.transpose(ps_ut[:, q * NAT:(q + 1) * NAT],
                                        ucm_l[ci][:, q * 128:(q + 1) * 128],
                                        ident[0:NAT, 0:NAT])
                nc.scalar.copy(
                    utok[:, qb:qb + nq, :],
                    bass.AP(tensor=ps_ut.tensor, offset=ps_ut.offset,
                            ap=[list(ps_ut.ap[0]), [NAT, nq], [1, NAT]]))
                ps_dt = psT.tile([128, 4 * 96], f32, tag="tp")
                for q in range(nq):
                    nc.tensor.transpose(ps_dt[:, q * 96:(q + 1) * 96],
                                        dcm_l[ci][:, q * 128:(q + 1) * 128],
                                        ident[0:96, 0:96])
                nc.vector.tensor_copy(
                    drtok[:, qb:qb + nq, :],
                    bass.AP(tensor=ps_dt.tensor, offset=ps_dt.offset,
                            ap=[list(ps_dt.ap[0]), [96, nq], [1, 96]]))
            return utok, drtok

        def emit_back(t0, gg, utok, drtok):
            TS = gg * 128
            rot_t = big.tile([128, gg, 9], f32, tag="rot")
            nc.sync.dma_start(rot_t,
                              rot_d[t0:t0 + TS].rearrange("(g p) c -> p g c", p=128))
            tr_t = big.tile([128, gg, 3], f32, tag="tr")
            nc.sync.dma_start(tr_t,
                              tr_d[t0:t0 + TS].rearrange("(g p) c -> p g c", p=128))

            # --- angles ---
            sq_t = tmp.tile([128, gg, NAT], f32, tag="t224")
            nc.vector.tensor_tensor(sq_t, utok, utok, op=OP.mult)
            ps7 = tmp.tile([128, gg, 7], f32, tag="t112a")
            nc.vector.tensor_tensor(ps7, sq_t[:, :, 0:NAT:2], sq_t[:, :, 1:NAT:2],
                                    op=OP.add)
            nc.vector.tensor_scalar_max(ps7, ps7, EPS)
            sr7 = tmp.tile([128, gg, 7], f32, tag="t112b")
            nc.scalar.activation(sr7, ps7, AF.Sqrt)
            rn7 = tmp.tile([128, gg, 7], f32, tag="t112c")
            nc.vector.reciprocal(rn7, sr7)
            sincos = big.tile([128, gg, 16], f32, tag="sincos")
            nc.gpsimd.memset(sincos[:, :, 0:1], 0.0)
            nc.gpsimd.memset(sincos[:, :, 1:2], 1.0)
            nc.vector.tensor_tensor(_cv(sincos, 2, [(2, 7), (1, 2)]),
                                    _cv(utok, 0, [(2, 7), (1, 2)]),
                                    _cv(rn7, 0, [(1, 7), (0, 2)]), op=OP.mult)
            nc.sync.dma_start(u_o[t0:t0 + TS].rearrange("(g p) c -> p g c", p=128),
                              utok)
            nc.sync.dma_start(ang_o[t0:t0 + TS].rearrange("(g p) c -> p g c", p=128),
                              sincos[:, :, 2:16])

            # --- rotx compose: fR = dR @ rotx(angle) ---
            Rbb = big.tile([128, gg, 72], f32, tag="Rbb")
            fRt = big.tile([128, gg, 27], f32, tag="fRt")
            tbb = big.tile([128, gg, 24], f32, tag="tbb")

            def rotx(dst, doff, fs, cnt, eng):
                dv = lambda j, st=9: _cv(dst, doff + j, [(st, cnt), (3, 3)])
                sv = lambda j: _cv(drtok, fs * 9 + j, [(9, cnt), (3, 3)])
                cb = _cv(sincos, fs * 2 + 1, [(2, cnt), (0, 3)])
                sb_ = _cv(sincos, fs * 2 + 0, [(2, cnt), (0, 3)])
                eng.tensor_copy(dv(0), sv(0))
                ta = tmp.tile([128, gg, cnt * 3], f32, tag="t240a")
                tav = _cv(ta, 0, [(3, cnt), (1, 3)])
                tb = tmp.tile([128, gg, cnt * 3], f32, tag="t240b")
                tbv = _cv(tb, 0, [(3, cnt), (1, 3)])
                eng.tensor_tensor(tav, sv(1), cb, op=OP.mult)
                eng.tensor_tensor(tbv, sv(2), sb_, op=OP.mult)
                eng.tensor_tensor(dv(1), tav, tbv, op=OP.add)
                eng.tensor_tensor(tav, sv(2), cb, op=OP.mult)
                eng.tensor_tensor(tbv, sv(1), sb_, op=OP.mult)
                eng.tensor_tensor(dv(2), tav, tbv, op=OP.subtract)

            rotx(Rbb, 0, 0, 5, nc.vector)
            rotx(fRt, 0, 5, 3, nc.gpsimd)
            nc.gpsimd.tensor_copy(_cv(tbb, 0, [(1, 15)]), _cv(drtok, 72, [(1, 15)]))

            # --- chain composes: frames 5,6,7 ---
            for kidx, f in enumerate([5, 6, 7]):
                af = 4 + kidx          # A frame slot in Rbb/tbb
                df = 5 + kidx          # dest slot
                soff = (f - 5) * 9     # source in fRt
                DV = _cv(Rbb, df * 9, [(3, 3), (1, 3)])
                t9 = tmp.tile([128, gg, 9], f32, tag="t144")
                t9v = _cv(t9, 0, [(3, 3), (1, 3)])
                for k in range(3):
                    Ak = _cv(Rbb, af * 9 + k, [(3, 3), (0, 3)])
                    Bk = _cv(fRt, soff + k * 3, [(0, 3), (1, 3)])
                    if k == 0:
                        nc.vector.tensor_tensor(DV, Ak, Bk, op=OP.mult)
                    else:
                        nc.vector.tensor_tensor(t9v, Ak, Bk, op=OP.mult)
                        nc.vector.tensor_tensor(DV, DV, t9v, op=OP.add)
                tv = _cv(tbb, df * 3, [(1, 3)])
                t3 = tmp.tile([128, gg, 3], f32, tag="t48")
                t3v = _cv(t3, 0, [(1, 3)])
                for k in range(3):
                    Ak = _cv(Rbb, af * 9 + k, [(3, 3)])
                    Btk = _cv(drtok, 72 + f * 3 + k, [(0, 3)])
                    if k == 0:
                        nc.gpsimd.tensor_tensor(tv, Ak, Btk, op=OP.mult)
                    else:
                        nc.gpsimd.tensor_tensor(t3v, Ak, Btk, op=OP.mult)
                        nc.gpsimd.tensor_tensor(tv, tv, t3v, op=OP.add)
                nc.gpsimd.tensor_tensor(tv, tv, _cv(tbb, af * 3, [(1, 3)]),
                                        op=OP.add)

            # --- backbone compose into sc44 layout ---
            sc44 = big.tile([128, gg, 128], f32, tag="sc44")
            nc.gpsimd.memset(_cv(sc44, 12, [(16, 8), (1, 3)]), 0.0)
            nc.gpsimd.memset(_cv(sc44, 15, [(16, 8)]), 1.0)
            t24 = tmp.tile([128, gg, 24], f32, tag="t24")
            t24v = _cv(t24, 0, [(3, 8), (1, 3)])
            for i in range(3):
                for k in range(3):
                    in0 = _cv(Rbb, k * 3, [(9, 8), (1, 3)])
                    in1 = _cv(rot_t, i * 3 + k, [(0, 8), (0, 3)])
                    dst = _cv(sc44, i * 4, [(16, 8), (1, 3)])
                    if k == 0:
                        nc.vector.tensor_tensor(dst, in0, in1, op=OP.mult)
                    else:
                        nc.vector.tensor_tensor(t24v, in0, in1, op=OP.mult)
                        nc.vector.tensor_tensor(dst, dst, t24v, op=OP.add)
            t8 = tmp.tile([128, gg, 8], f32, tag="t8")
            t8v = _cv(t8, 0, [(1, 8)])
            for i in range(3):
                dst = _cv(sc44, i * 4 + 3, [(16, 8)])
                for k in range(3):
                    in0 = _cv(tbb, k, [(3, 8)])
                    in1 = _cv(rot_t, i * 3 + k, [(0, 8)])
                    if k == 0:
                        nc.gpsimd.tensor_tensor(dst, in0, in1, op=OP.mult)
                    else:
                        nc.gpsimd.tensor_tensor(t8v, in0, in1, op=OP.mult)
                        nc.gpsimd.tensor_tensor(dst, dst, t8v, op=OP.add)
                nc.gpsimd.tensor_tensor(dst, dst, _cv(tr_t, i, [(0, 8)]),
                                        op=OP.add)
            nc.sync.dma_start(sc_o[t0:t0 + TS].rearrange("(g p) c -> p g c", p=128),
                              sc44)

            # --- per 512-token window: transpose sc44 to channel-major,
            # per-class pos matmuls, transpose pos back to token-major ---
            postok = big.tile([128, gg, 42], f32, tag="postok")
            nbo = 0
            while nbo < TS:
                w = min(512, TS - nbo)
                nq = w // 128
                qb = nbo // 128
                wlo, whi = t0 + nbo, t0 + nbo + w
                ps_rt = psT.tile([128, 512], f32, tag="tp")
                for q2 in range(nq):
                    nc.tensor.transpose(ps_rt[:, q2 * 128:(q2 + 1) * 128],
                                        sc44[:, qb + q2, :], ident)
                rtc_t = rp4.tile([128, w], f32, tag="rtc")
                nc.scalar.copy(rtc_t, ps_rt[:, 0:w])
                ps_pos = psT.tile([42, 512], f32, tag="tp")
                for (r, lo, hi) in runs:
                    a, b_ = max(lo, wlo), min(hi, whi)
                    if a >= b_:
                        continue
                    nc.tensor.matmul(ps_pos[:, a - wlo:b_ - wlo],
                                     pm_sb[:, r, :], rtc_t[:, a - wlo:b_ - wlo],
                                     start=True, stop=True)
                poscm_t = rp4.tile([42, w], f32, tag="poscm")
                nc.vector.tensor_copy(poscm_t, ps_pos[:, 0:w])
                ps_pt = psT.tile([128, 4 * 42], f32, tag="tp")
                for q2 in range(nq):
                    nc.tensor.transpose(ps_pt[:, q2 * 42:(q2 + 1) * 42],
                                        poscm_t[:, q2 * 128:(q2 + 1) * 128],
                                        ident[0:42, 0:42])
                nc.vector.tensor_copy(
                    postok[:, qb:qb + nq, :],
                    bass.AP(tensor=ps_pt.tensor, offset=ps_pt.offset,
                            ap=[list(ps_pt.ap[0]), [42, nq], [1, 42]]))
                nbo += w
            nc.sync.dma_start(pos_o[t0:t0 + TS].rearrange("(g p) c -> p g c", p=128),
                              postok)

        # ---- software-pipelined emission: front(k+1) before back(k) so the
        # PE instruction stream never stalls long on DVE structure output ----
        sbs = []
        t0 = 0
        while t0 < TDEV:
            gg = min(SBT, NT - t0 // 128)
            sbs.append((t0, gg))
            t0 += gg * 128
        pend = None
        for (s0, sg) in sbs:
            ut, dr = emit_front(s0, sg)
            if pend is not None:
                emit_back(*pend)
            pend = (s0, sg, ut, dr)
        emit_back(*pend)

    nc.compile()
    return nc


# ---------------- host staging ----------------

def _tf32_round(a):
    """Round-to-nearest-even to f32r precision (8-bit exp, 11-bit mantissa)."""
    b = np.ascontiguousarray(a, np.float32).view(np.uint32)
    lsb = (b >> 12) & 1
    r = (b + 0x7FF + lsb) & np.uint32(0xFFFFF000)
    return r.view(np.float32)


def _stage(inputs, use_f32r=True):
    s = np.ascontiguousarray(inputs["s"], np.float32)
    si = np.ascontiguousarray(inputs["s_initial"], np.float32)
    rot = np.ascontiguousarray(inputs["rot_mats"], np.float32)
    tr = np.ascontiguousarray(inputs["trans"], np.float32)
    aat = np.asarray(inputs["aatype"]).astype(np.int32)
    df = np.ascontiguousarray(inputs["default_frames"], np.float32)
    gidx = np.asarray(inputs["group_idx"]).astype(np.int64)
    amask = np.ascontiguousarray(inputs["atom_mask"], np.float32)
    lit = np.ascontiguousarray(inputs["lit_positions"], np.float32)

    counts = np.zeros((NCORE, NRES), np.int64)
    aat_c = aat.reshape(NCORE, TC)
    for c in range(NCORE):
        counts[c] = np.bincount(aat_c[c], minlength=NRES)
    caps = (np.ceil(counts.max(0) / 128).astype(int) * 128).clip(128)
    TDEV = int(caps.sum())
    offs = np.concatenate([[0], np.cumsum(caps)]).astype(int)

    # tables
    dR = df[:, :, :3, :3].reshape(NRES, 72)
    dt = df[:, :, :3, 3].reshape(NRES, 24)
    dfl = np.ascontiguousarray(np.concatenate([dR, dt], 1), np.float32)
    pmw = np.zeros((NRES, 128, 42), np.float32)
    rr, aa_ = np.meshgrid(np.arange(NRES), np.arange(NAT), indexing="ij")
    g_of = gidx  # [21,14]
    for i in range(3):
        for jj in range(4):
            val = amask * (lit[:, :, jj] if jj < 3 else 1.0)
            pmw[rr, g_of * 16 + i * 4 + jj, aa_ * 3 + i] = val

    w_init = np.ascontiguousarray(inputs["w_init"], np.float32)
    w_in = np.ascontiguousarray(inputs["w_in"], np.float32)
    wi3 = np.ascontiguousarray(w_init.reshape(3, 128, CH))
    wn3 = np.ascontiguousarray(w_in.reshape(3, 128, CH))
    bw1 = np.ascontiguousarray(inputs["blk_w1"], np.float32)
    bw2 = np.ascontiguousarray(inputs["blk_w2"], np.float32)
    wo = np.ascontiguousarray(inputs["w_out"], np.float32)
    b_init = np.asarray(inputs["b_init"], np.float32)
    b_in = np.asarray(inputs["b_in"], np.float32)
    bb1 = np.ascontiguousarray(inputs["blk_b1"], np.float32)
    bb2 = np.ascontiguousarray(inputs["blk_b2"], np.float32)
    bo = np.asarray(inputs["b_out"], np.float32)
    has_bias = bool(np.any(b_init) or np.any(b_in) or np.any(bb1)
                    or np.any(bb2) or np.any(bo))

    if use_f32r:
        wi3, wn3, bw1, bw2, wo = (_tf32_round(x) for x in (wi3, wn3, bw1, bw2, wo))
    shared = dict(dfl=dfl, pmw=pmw, wi3=wi3, wn3=wn3, bw1=bw1, bw2=bw2, wo=wo)
    if has_bias:
        shared.update(bcomb=(b_init + b_in).astype(np.float32), bb1=bb1,
                      bb2=bb2, bo=bo)

    in_maps, slot_maps = [], []
    s2 = s.reshape(NCORE, TC, CS)
    si2 = si.reshape(NCORE, TC, CS)
    rot2 = rot.reshape(NCORE, TC, 9)
    tr2 = tr.reshape(NCORE, TC, 3)
    for c in range(NCORE):
        a = aat_c[c]
        order = np.argsort(a, kind="stable")
        cnt = counts[c]
        # slot index for each source token
        rank = np.empty(TC, np.int64)
        rank[order] = np.arange(TC)
        csum = np.concatenate([[0], np.cumsum(cnt)])
        slot_of_src = offs[a] + (rank - csum[a])
        sT = np.zeros((CS, TDEV), np.float32)
        sT[:, slot_of_src] = np.maximum(s2[c], 0).T
        siT = np.zeros((CS, TDEV), np.float32)
        siT[:, slot_of_src] = np.maximum(si2[c], 0).T
        if use_f32r:
            sT = _tf32_round(sT)
            siT = _tf32_round(siT)
        rot9 = np.zeros((TDEV, 9), np.float32)
        rot9[slot_of_src] = rot2[c]
        tr3 = np.zeros((TDEV, 3), np.float32)
        tr3[slot_of_src] = tr2[c] * TRANS_SCALE
        oneh = np.zeros((NRES, TDEV), np.float32)
        oneh[a, slot_of_src] = 1.0
        m = dict(shared)
        m.update(sT=sT, siT=siT, rot9=rot9, tr3=tr3, oneh=oneh)
        in_maps.append(m)
        slot_maps.append(slot_of_src)
    return in_maps, slot_maps, tuple(int(x) for x in caps), has_bias


def _frames7(rot_mats, trans):
    import jax
    import jax.numpy as jnp
    cpu = jax.devices("cpu")[0]
    with jax.default_device(cpu):
        rot = jnp.asarray(rot_mats)
        xx, xy, xz = rot[..., 0, 0], rot[..., 0, 1], rot[..., 0, 2]
        yx, yy, yz = rot[..., 1, 0], rot[..., 1, 1], rot[..., 1, 2]
        zx, zy, zz = rot[..., 2, 0], rot[..., 2, 1], rot[..., 2, 2]
        k = jnp.stack([
            jnp.stack([xx + yy + zz, zy - yz, xz - zx, yx - xy], -1),
            jnp.stack([zy - yz, xx - yy - zz, xy + yx, xz + zx], -1),
            jnp.stack([xz - zx, xy + yx, yy - xx - zz, yz + zy], -1),
            jnp.stack([yx - xy, xz + zx, yz + zy, zz - xx - yy], -1),
        ], -2) / 3.0
        _, vecs = jnp.linalg.eigh(k)
        quat = vecs[..., -1]
        bb_t = jnp.asarray(trans) * TRANS_SCALE
        out = jnp.concatenate([quat, bb_t], axis=-1)
        return np.asarray(out)


_CACHE = {}
TRACE = False
USE_F32R = False
LAST_RESULTS = None


def kernel(**inputs):
    global LAST_RESULTS
    in_maps, slot_maps, caps, has_bias = _stage(inputs, USE_F32R)
    key = (caps, has_bias, USE_F32R)
    if key not in _CACHE:
        _CACHE[key] = build_nc(list(caps), has_bias, USE_F32R)
    nc = _CACHE[key]
    rr = run_bass_kernel_spmd(nc, in_maps, core_ids=list(range(NCORE)),
                              trace=TRACE)
    LAST_RESULTS = rr
    res = rr.results

    sc44 = np.empty((B, N, 8, 4, 4), np.float32)
    unnorm = np.empty((B, N, 7, 2), np.float32)
    angles = np.empty((B, N, 7, 2), np.float32)
    pos = np.empty((B, N, NAT, 3), np.float32)
    for c in range(NCORE):
        sl = slot_maps[c]
        bs = slice(c * MC, (c + 1) * MC)
        sc44[bs] = res[c]["sc44_o"][sl].reshape(MC, N, 8, 4, 4)
        unnorm[bs] = res[c]["u_o"][sl].reshape(MC, N, 7, 2)
        angles[bs] = res[c]["ang_o"][sl].reshape(MC, N, 7, 2)
        pos[bs] = res[c]["pos_o"][sl].reshape(MC, N, NAT, 3)

    frames7 = _frames7(np.asarray(inputs["rot_mats"], np.float32),
                       np.asarray(inputs["trans"], np.float32))
    s_out = np.asarray(inputs["s"], np.float32)
    return frames7, sc44, unnorm, angles, pos, s_out


# revision 11
# speedup vs baseline: 1.8158x; 1.1608x over previous
"""Trainium2 Bass kernel for the AlphaFold-style structure module.

Self-contained: hardcodes shapes/sharding. kernel(**inputs) -> tuple of outputs
matching the reference (frames7, sc44, unnorm, angles, pos, s).
"""
import numpy as np
from contextlib import ExitStack

import concourse.bass as bass
import concourse.mybir as mybir
import concourse.tile as tile
from concourse import bacc
from concourse.bass_utils import run_bass_kernel_spmd
from concourse.masks import make_identity

f32 = mybir.dt.float32
f32r = mybir.dt.float32r
AF = mybir.ActivationFunctionType
OP = mybir.AluOpType

B, N, CS, CH = 128, 512, 384, 128
NCORE = 8
MC = B // NCORE            # members per core (16)
TC = MC * N                # tokens per core (8192)
NFR, NAT, NRES = 8, 14, 21
EPS = 1e-8
TRANS_SCALE = 10.0
SBT = 16                   # token tiles per superblock


def _r(t):
    return t


def _cv(t, off, dims):
    """Free-dim view of a 3D tile [128, gg, C]: keeps partition+group dims,
    replaces comp dim with custom (step, count) dims at element offset off."""
    return bass.AP(tensor=t.tensor, offset=t.offset + off,
                   ap=[list(t.ap[0]), list(t.ap[1])] + [list(d) for d in dims])


def build_nc(caps, has_bias, use_f32r=True):
    """Emit the full Tile program for one core. caps: list of 21 per-class
    slot counts (multiples of 128)."""
    MDT = f32r if use_f32r else f32
    TDEV = int(sum(caps))
    assert TDEV % 128 == 0
    NT = TDEV // 128
    offs = np.concatenate([[0], np.cumsum(caps)]).astype(int)

    nc = bacc.Bacc("TRN2", target_bir_lowering=False, debug=False,
                   num_devices=NCORE)

    # ---- DRAM tensors ----
    sT_d = nc.dram_tensor("sT", [CS, TDEV], MDT, kind="ExternalInput").ap()
    siT_d = nc.dram_tensor("siT", [CS, TDEV], MDT, kind="ExternalInput").ap()
    rot_d = nc.dram_tensor("rot9", [TDEV, 9], f32, kind="ExternalInput").ap()
    tr_d = nc.dram_tensor("tr3", [TDEV, 3], f32, kind="ExternalInput").ap()
    dtk_d = nc.dram_tensor("dfl_tok", [TDEV, 96], f32, kind="ExternalInput").ap()
    pm_d = nc.dram_tensor("pmw", [NRES, 128, 42], f32, kind="ExternalInput").ap()
    wi_d = nc.dram_tensor("wi3", [3, CH, CH], MDT, kind="ExternalInput").ap()
    wn_d = nc.dram_tensor("wn3", [3, CH, CH], MDT, kind="ExternalInput").ap()
    b1w_d = nc.dram_tensor("bw1", [2, CH, CH], MDT, kind="ExternalInput").ap()
    b2w_d = nc.dram_tensor("bw2", [2, CH, CH], MDT, kind="ExternalInput").ap()
    wo_d = nc.dram_tensor("wo", [CH, NAT], MDT, kind="ExternalInput").ap()
    if has_bias:
        bc_d = nc.dram_tensor("bcomb", [CH], f32, kind="ExternalInput").ap()
        bb1_d = nc.dram_tensor("bb1", [2, CH], f32, kind="ExternalInput").ap()
        bb2_d = nc.dram_tensor("bb2", [2, CH], f32, kind="ExternalInput").ap()
        bo_d = nc.dram_tensor("bo", [NAT], f32, kind="ExternalInput").ap()

    sc_o = nc.dram_tensor("sc44_o", [TDEV, 128], f32, kind="ExternalOutput").ap()
    u_o = nc.dram_tensor("u_o", [TDEV, NAT], f32, kind="ExternalOutput").ap()
    ang_o = nc.dram_tensor("ang_o", [TDEV, NAT], f32, kind="ExternalOutput").ap()
    pos_o = nc.dram_tensor("pos_o", [42, TDEV], f32, kind="ExternalOutput").ap()

    with tile.TileContext(nc) as tc, ExitStack() as ctx:
        wp = ctx.enter_context(tc.tile_pool(name="wp", bufs=1))
        stp = ctx.enter_context(tc.tile_pool(name="stp", bufs=4))
        rp = ctx.enter_context(tc.tile_pool(name="rp", bufs=5))
        rp4 = ctx.enter_context(tc.tile_pool(name="rp4", bufs=4))
        big = ctx.enter_context(tc.tile_pool(name="big", bufs=2))
        tmp = ctx.enter_context(tc.tile_pool(name="tmp", bufs=2))
        psA = ctx.enter_context(tc.tile_pool(name="psA", bufs=4, space="PSUM"))
        psU = ctx.enter_context(tc.tile_pool(name="psU", bufs=1, space="PSUM"))
        psT = ctx.enter_context(tc.tile_pool(name="psT", bufs=2, space="PSUM"))

        # ---- preload constants ----
        ident = wp.tile([128, 128], f32)
        make_identity(nc, ident)
        wi_sb = wp.tile([CH, 3, CH], MDT, tag="wi")
        nc.sync.dma_start(wi_sb, wi_d.rearrange("c p m -> p c m"))
        wn_sb = wp.tile([CH, 3, CH], MDT, tag="wn")
        nc.sync.dma_start(wn_sb, wn_d.rearrange("c p m -> p c m"))
        b1_sb = wp.tile([CH, 2, CH], MDT, tag="b1w")
        nc.sync.dma_start(b1_sb, b1w_d.rearrange("c p m -> p c m"))
        b2_sb = wp.tile([CH, 2, CH], MDT, tag="b2w")
        nc.sync.dma_start(b2_sb, b2w_d.rearrange("c p m -> p c m"))
        wo_sb = wp.tile([CH, NAT], MDT, tag="wo")
        nc.sync.dma_start(wo_sb, wo_d)
        pm_sb = wp.tile([128, NRES, 42], f32, tag="pm")
        nc.sync.dma_start(pm_sb, pm_d.rearrange("r k m -> k r m"))
        if has_bias:
            bc_sb = wp.tile([CH, 1], f32, tag="bc")
            nc.sync.dma_start(bc_sb, bc_d[:, None])
            bb1_sb = wp.tile([CH, 2], f32, tag="bb1")
            nc.sync.dma_start(bb1_sb, bb1_d.rearrange("i c -> c i"))
            bb2_sb = wp.tile([CH, 2], f32, tag="bb2")
            nc.sync.dma_start(bb2_sb, bb2_d.rearrange("i c -> c i"))
            bo_sb = wp.tile([NAT, 1], f32, tag="bo")
            nc.sync.dma_start(bo_sb, bo_d[:, None])

        sT3 = sT_d.rearrange("(c p) t -> p c t", c=3)
        siT3 = siT_d.rearrange("(c p) t -> p c t", c=3)

        # global class runs [(r, lo, hi)) in device token space
        runs = [(r, int(offs[r]), int(offs[r + 1])) for r in range(NRES)
                if offs[r + 1] > offs[r]]

        def emit_front(t0, gg):
            TS = gg * 128
            utok = big.tile([128, gg, NAT], f32, tag="utok")
            drtok = big.tile([128, gg, 96], f32, tag="drtok")
            nc.sync.dma_start(drtok,
                              dtk_d[t0:t0 + TS].rearrange("(g p) c -> p g c", p=128))
            chunks = []
            nbo = 0
            while nbo < TS:
                w = min(512, TS - nbo)
                chunks.append((nbo, w))
                nbo += w
            # wave 0: DMAs
            st_l, si_l = [], []
            for (nbo, w) in chunks:
                sl = slice(t0 + nbo, t0 + nbo + w)
                st_t = stp.tile([128, 3, w], MDT, tag="st")
                nc.sync.dma_start(st_t, sT3[:, :, sl])
                si_t = stp.tile([128, 3, w], MDT, tag="si")
                nc.sync.dma_start(si_t, siT3[:, :, sl])
                st_l.append(st_t); si_l.append(si_t)
            # wave 1: L1 matmuls
            psx_l = []
            for ci, (nbo, w) in enumerate(chunks):
                ps_x = psA.tile([128, w], f32, tag="mm")
                for c in range(3):
                    nc.tensor.matmul(ps_x, _r(wi_sb[:, c, :]), _r(si_l[ci][:, c, :]),
                                     start=(c == 0), stop=False)
                for c in range(3):
                    nc.tensor.matmul(ps_x, _r(wn_sb[:, c, :]), _r(st_l[ci][:, c, :]),
                                     start=False, stop=(c == 2))
                psx_l.append(ps_x)
            # wave 2: x copy + relu
            x_l, r_l = [], []
            for ci, (nbo, w) in enumerate(chunks):
                x_t = rp.tile([128, w], f32, tag="x")
                r_t = rp.tile([128, w], MDT, tag="r")
                if has_bias:
                    nc.vector.tensor_scalar_add(x_t, psx_l[ci], bc_sb[:, 0:1])
                    nc.scalar.activation(r_t, psx_l[ci], AF.Relu, bias=bc_sb[:, 0:1])
                else:
                    nc.scalar.copy(x_t, psx_l[ci])
                    nc.scalar.activation(r_t, psx_l[ci], AF.Relu)
                x_l.append(x_t); r_l.append(r_t)
            # residual blocks, wave per layer
            for i in range(2):
                psh_l = []
                for ci, (nbo, w) in enumerate(chunks):
                    ps_h = psA.tile([128, w], f32, tag="mm")
                    nc.tensor.matmul(ps_h, _r(b1_sb[:, i, :]), _r(r_l[ci]))
                    psh_l.append(ps_h)
                r2_l = []
                for ci, (nbo, w) in enumerate(chunks):
                    r2_t = rp4.tile([128, w], MDT, tag="r2")
                    if has_bias:
                        nc.scalar.activation(r2_t, psh_l[ci], AF.Relu,
                                             bias=bb1_sb[:, i:i + 1])
                    else:
                        nc.scalar.activation(r2_t, psh_l[ci], AF.Relu)
                    r2_l.append(r2_t)
                psh2_l = []
                for ci, (nbo, w) in enumerate(chunks):
                    ps_h2 = psA.tile([128, w], f32, tag="mm")
                    nc.tensor.matmul(ps_h2, _r(b2_sb[:, i, :]), _r(r2_l[ci]))
                    psh2_l.append(ps_h2)
                for ci, (nbo, w) in enumerate(chunks):
                    x2_t = rp.tile([128, w], f32, tag="x")
                    if has_bias:
                        nc.vector.scalar_tensor_tensor(
                            x2_t, psh2_l[ci], bb2_sb[:, i:i + 1], x_l[ci],
                            op0=OP.add, op1=OP.add)
                    else:
                        nc.vector.tensor_tensor(x2_t, psh2_l[ci], x_l[ci],
                                                op=OP.add)
                    r_t = rp.tile([128, w], MDT, tag="r")
                    nc.vector.tensor_scalar_max(r_t, x2_t, 0.0)
                    x_l[ci] = x2_t; r_l[ci] = r_t
            # out layer + gathers
            psu_l = []
            for ci, (nbo, w) in enumerate(chunks):
                ps_u = psU.tile([NAT, w], f32, tag="u")
                nc.tensor.matmul(ps_u, _r(wo_sb), _r(r_l[ci]))
                psu_l.append(ps_u)
            ucm_l = []
            for ci, (nbo, w) in enumerate(chunks):
                ucm_t = rp4.tile([NAT, w], f32, tag="ucm")
                if has_bias:
                    nc.vector.tensor_scalar_add(ucm_t, psu_l[ci], bo_sb[:, 0:1])
                else:
                    nc.scalar.copy(ucm_t, psu_l[ci])
                ucm_l.append(ucm_t)
            # transposes to token-major
            for ci, (nbo, w) in enumerate(chunks):
                nq = w // 128
                qb = nbo // 128
                ps_ut = psT.tile([128, 4 * NAT], f32, tag="tp")
                for q in range(nq):
                    nc.tensor.transpose(ps_ut[:, q * NAT:(q + 1) * NAT],
                                        ucm_l[ci][:, q * 128:(q + 1) * 128],
                                        ident[0:NAT, 0:NAT])
                nc.scalar.copy(
                    utok[:, qb:qb + nq, :],
                    bass.AP(tensor=ps_ut.tensor, offset=ps_ut.offset,
                            ap=[list(ps_ut.ap[0]), [NAT, nq], [1, NAT]]))
            return utok, drtok

        def emit_back(t0, gg, utok, drtok):
            TS = gg * 128
            rot_t = big.tile([128, gg, 9], f32, tag="rot")
            nc.sync.dma_start(rot_t,
                              rot_d[t0:t0 + TS].rearrange("(g p) c -> p g c", p=128))
            tr_t = big.tile([128, gg, 3], f32, tag="tr")
            nc.sync.dma_start(tr_t,
                              tr_d[t0:t0 + TS].rearrange("(g p) c -> p g c", p=128))

            # --- angles ---
            sq_t = tmp.tile([128, gg, NAT], f32, tag="t224")
            nc.vector.tensor_tensor(sq_t, utok, utok, op=OP.mult)
            ps7 = tmp.tile([128, gg, 7], f32, tag="t112a")
            nc.vector.tensor_tensor(ps7, sq_t[:, :, 0:NAT:2], sq_t[:, :, 1:NAT:2],
                                    op=OP.add)
            nc.vector.tensor_scalar_max(ps7, ps7, EPS)
            sr7 = tmp.tile([128, gg, 7], f32, tag="t112b")
            nc.scalar.activation(sr7, ps7, AF.Sqrt)
            rn7 = tmp.tile([128, gg, 7], f32, tag="t112c")
            nc.vector.reciprocal(rn7, sr7)
            sincos = big.tile([128, gg, 16], f32, tag="sincos")
            nc.gpsimd.memset(sincos[:, :, 0:1], 0.0)
            nc.gpsimd.memset(sincos[:, :, 1:2], 1.0)
            nc.vector.tensor_tensor(_cv(sincos, 2, [(2, 7), (1, 2)]),
                                    _cv(utok, 0, [(2, 7), (1, 2)]),
                                    _cv(rn7, 0, [(1, 7), (0, 2)]), op=OP.mult)
            nc.sync.dma_start(u_o[t0:t0 + TS].rearrange("(g p) c -> p g c", p=128),
                              utok)
            nc.sync.dma_start(ang_o[t0:t0 + TS].rearrange("(g p) c -> p g c", p=128),
                              sincos[:, :, 2:16])

            # --- rotx compose: fR = dR @ rotx(angle) ---
            Rbb = big.tile([128, gg, 72], f32, tag="Rbb")
            fRt = big.tile([128, gg, 27], f32, tag="fRt")
            tbb = big.tile([128, gg, 24], f32, tag="tbb")

            def rotx(dst, doff, fs, cnt, eng):
                dv = lambda j, st=9: _cv(dst, doff + j, [(st, cnt), (3, 3)])
                sv = lambda j: _cv(drtok, fs * 9 + j, [(9, cnt), (3, 3)])
                cb = _cv(sincos, fs * 2 + 1, [(2, cnt), (0, 3)])
                sb_ = _cv(sincos, fs * 2 + 0, [(2, cnt), (0, 3)])
                eng.tensor_copy(dv(0), sv(0))
                ta = tmp.tile([128, gg, cnt * 3], f32, tag="t240a")
                tav = _cv(ta, 0, [(3, cnt), (1, 3)])
                tb = tmp.tile([128, gg, cnt * 3], f32, tag="t240b")
                tbv = _cv(tb, 0, [(3, cnt), (1, 3)])
                eng.tensor_tensor(tav, sv(1), cb, op=OP.mult)
                eng.tensor_tensor(tbv, sv(2), sb_, op=OP.mult)
                eng.tensor_tensor(dv(1), tav, tbv, op=OP.add)
                eng.tensor_tensor(tav, sv(2), cb, op=OP.mult)
                eng.tensor_tensor(tbv, sv(1), sb_, op=OP.mult)
                eng.tensor_tensor(dv(2), tav, tbv, op=OP.subtract)

            rotx(Rbb, 0, 0, 5, nc.vector)
            rotx(fRt, 0, 5, 3, nc.gpsimd)
            nc.gpsimd.tensor_copy(_cv(tbb, 0, [(1, 15)]), _cv(drtok, 72, [(1, 15)]))

            # --- chain composes: frames 5,6,7 ---
            for kidx, f in enumerate([5, 6, 7]):
                af = 4 + kidx          # A frame slot in Rbb/tbb
                df = 5 + kidx          # dest slot
                soff = (f - 5) * 9     # source in fRt
                DV = _cv(Rbb, df * 9, [(3, 3), (1, 3)])
                t9 = tmp.tile([128, gg, 9], f32, tag="t144")
                t9v = _cv(t9, 0, [(3, 3), (1, 3)])
                for k in range(3):
                    Ak = _cv(Rbb, af * 9 + k, [(3, 3), (0, 3)])
                    Bk = _cv(fRt, soff + k * 3, [(0, 3), (1, 3)])
                    if k == 0:
                        nc.vector.tensor_tensor(DV, Ak, Bk, op=OP.mult)
                    else:
                        nc.vector.tensor_tensor(t9v, Ak, Bk, op=OP.mult)
                        nc.vector.tensor_tensor(DV, DV, t9v, op=OP.add)
                tv = _cv(tbb, df * 3, [(1, 3)])
                t3 = tmp.tile([128, gg, 3], f32, tag="t48")
                t3v = _cv(t3, 0, [(1, 3)])
                for k in range(3):
                    Ak = _cv(Rbb, af * 9 + k, [(3, 3)])
                    Btk = _cv(drtok, 72 + f * 3 + k, [(0, 3)])
                    if k == 0:
                        nc.gpsimd.tensor_tensor(tv, Ak, Btk, op=OP.mult)
                    else:
                        nc.gpsimd.tensor_tensor(t3v, Ak, Btk, op=OP.mult)
                        nc.gpsimd.tensor_tensor(tv, tv, t3v, op=OP.add)
                nc.gpsimd.tensor_tensor(tv, tv, _cv(tbb, af * 3, [(1, 3)]),
                                        op=OP.add)

            # --- backbone compose into sc44 layout ---
            sc44 = big.tile([128, gg, 128], f32, tag="sc44")
            nc.gpsimd.memset(_cv(sc44, 12, [(16, 8), (1, 3)]), 0.0)
            nc.gpsimd.memset(_cv(sc44, 15, [(16, 8)]), 1.0)
            t24 = tmp.tile([128, gg, 24], f32, tag="t24")
            t24v = _cv(t24, 0, [(3, 8), (1, 3)])
            for i in range(3):
                for k in range(3):
                    in0 = _cv(Rbb, k * 3, [(9, 8), (1, 3)])
                    in1 = _cv(rot_t, i * 3 + k, [(0, 8), (0, 3)])
                    dst = _cv(sc44, i * 4, [(16, 8), (1, 3)])
                    if k == 0:
                        nc.vector.tensor_tensor(dst, in0, in1, op=OP.mult)
                    else:
                        nc.vector.tensor_tensor(t24v, in0, in1, op=OP.mult)
                        nc.vector.tensor_tensor(dst, dst, t24v, op=OP.add)
            t8 = tmp.tile([128, gg, 8], f32, tag="t8")
            t8v = _cv(t8, 0, [(1, 8)])
            for i in range(3):
                dst = _cv(sc44, i * 4 + 3, [(16, 8)])
                for k in range(3):
                    in0 = _cv(tbb, k, [(3, 8)])
                    in1 = _cv(rot_t, i * 3 + k, [(0, 8)])
                    if k == 0:
                        nc.gpsimd.tensor_tensor(dst, in0, in1, op=OP.mult)
                    else:
                        nc.gpsimd.tensor_tensor(t8v, in0, in1, op=OP.mult)
                        nc.gpsimd.tensor_tensor(dst, dst, t8v, op=OP.add)
                nc.gpsimd.tensor_tensor(dst, dst, _cv(tr_t, i, [(0, 8)]),
                                        op=OP.add)
            nc.sync.dma_start(sc_o[t0:t0 + TS].rearrange("(g p) c -> p g c", p=128),
                              sc44)

            # --- per 512-token window: transpose sc44 to channel-major,
            # per-class pos matmuls; pos stored channel-major (host transposes)
            nbo = 0
            while nbo < TS:
                w = min(512, TS - nbo)
                nq = w // 128
                qb = nbo // 128
                wlo, whi = t0 + nbo, t0 + nbo + w
                ps_rt = psT.tile([128, 512], f32, tag="tp")
                for q2 in range(nq):
                    nc.tensor.transpose(ps_rt[:, q2 * 128:(q2 + 1) * 128],
                                        sc44[:, qb + q2, :], ident)
                rtc_t = rp4.tile([128, w], f32, tag="rtc")
                nc.scalar.copy(rtc_t, ps_rt[:, 0:w])
                ps_pos = psT.tile([42, 512], f32, tag="tp")
                for (r, lo, hi) in runs:
                    a, b_ = max(lo, wlo), min(hi, whi)
                    if a >= b_:
                        continue
                    nc.tensor.matmul(ps_pos[:, a - wlo:b_ - wlo],
                                     pm_sb[:, r, :], rtc_t[:, a - wlo:b_ - wlo],
                                     start=True, stop=True)
                poscm_t = rp4.tile([42, w], f32, tag="poscm")
                nc.scalar.copy(poscm_t, ps_pos[:, 0:w])
                nc.sync.dma_start(pos_o[:, wlo:whi], poscm_t)
                nbo += w

        # ---- software-pipelined emission: front(k+1) before back(k) so the
        # PE instruction stream never stalls long on DVE structure output ----
        sbs = []
        t0 = 0
        while t0 < TDEV:
            gg = min(SBT, NT - t0 // 128)
            sbs.append((t0, gg))
            t0 += gg * 128
        pend = None
        for (s0, sg) in sbs:
            ut, dr = emit_front(s0, sg)
            if pend is not None:
                emit_back(*pend)
            pend = (s0, sg, ut, dr)
        emit_back(*pend)

    nc.compile()
    return nc


# ---------------- host staging ----------------

def _tf32_round(a):
    """Round-to-nearest-even to f32r precision (8-bit exp, 11-bit mantissa)."""
    b = np.ascontiguousarray(a, np.float32).view(np.uint32)
    lsb = (b >> 12) & 1
    r = (b + 0x7FF + lsb) & np.uint32(0xFFFFF000)
    return r.view(np.float32)


def _stage(inputs, use_f32r=True):
    s = np.ascontiguousarray(inputs["s"], np.float32)
    si = np.ascontiguousarray(inputs["s_initial"], np.float32)
    rot = np.ascontiguousarray(inputs["rot_mats"], np.float32)
    tr = np.ascontiguousarray(inputs["trans"], np.float32)
    aat = np.asarray(inputs["aatype"]).astype(np.int32)
    df = np.ascontiguousarray(inputs["default_frames"], np.float32)
    gidx = np.asarray(inputs["group_idx"]).astype(np.int64)
    amask = np.ascontiguousarray(inputs["atom_mask"], np.float32)
    lit = np.ascontiguousarray(inputs["lit_positions"], np.float32)

    counts = np.zeros((NCORE, NRES), np.int64)
    aat_c = aat.reshape(NCORE, TC)
    for c in range(NCORE):
        counts[c] = np.bincount(aat_c[c], minlength=NRES)
    caps = (np.ceil(counts.max(0) / 128).astype(int) * 128).clip(128)
    TDEV = int(caps.sum())
    offs = np.concatenate([[0], np.cumsum(caps)]).astype(int)

    # tables
    dR = df[:, :, :3, :3].reshape(NRES, 72)
    dt = df[:, :, :3, 3].reshape(NRES, 24)
    dfl = np.ascontiguousarray(np.concatenate([dR, dt], 1), np.float32)
    pmw = np.zeros((NRES, 128, 42), np.float32)
    rr, aa_ = np.meshgrid(np.arange(NRES), np.arange(NAT), indexing="ij")
    g_of = gidx  # [21,14]
    for i in range(3):
        for jj in range(4):
            val = amask * (lit[:, :, jj] if jj < 3 else 1.0)
            pmw[rr, g_of * 16 + i * 4 + jj, aa_ * 3 + i] = val

    w_init = np.ascontiguousarray(inputs["w_init"], np.float32)
    w_in = np.ascontiguousarray(inputs["w_in"], np.float32)
    wi3 = np.ascontiguousarray(w_init.reshape(3, 128, CH))
    wn3 = np.ascontiguousarray(w_in.reshape(3, 128, CH))
    bw1 = np.ascontiguousarray(inputs["blk_w1"], np.float32)
    bw2 = np.ascontiguousarray(inputs["blk_w2"], np.float32)
    wo = np.ascontiguousarray(inputs["w_out"], np.float32)
    b_init = np.asarray(inputs["b_init"], np.float32)
    b_in = np.asarray(inputs["b_in"], np.float32)
    bb1 = np.ascontiguousarray(inputs["blk_b1"], np.float32)
    bb2 = np.ascontiguousarray(inputs["blk_b2"], np.float32)
    bo = np.asarray(inputs["b_out"], np.float32)
    has_bias = bool(np.any(b_init) or np.any(b_in) or np.any(bb1)
                    or np.any(bb2) or np.any(bo))

    if use_f32r:
        wi3, wn3, bw1, bw2, wo = (_tf32_round(x) for x in (wi3, wn3, bw1, bw2, wo))
    shared = dict(pmw=pmw, wi3=wi3, wn3=wn3, bw1=bw1, bw2=bw2, wo=wo)
    if has_bias:
        shared.update(bcomb=(b_init + b_in).astype(np.float32), bb1=bb1,
                      bb2=bb2, bo=bo)

    in_maps, slot_maps = [], []
    s2 = s.reshape(NCORE, TC, CS)
    si2 = si.reshape(NCORE, TC, CS)
    rot2 = rot.reshape(NCORE, TC, 9)
    tr2 = tr.reshape(NCORE, TC, 3)
    for c in range(NCORE):
        a = aat_c[c]
        order = np.argsort(a, kind="stable")
        cnt = counts[c]
        # slot index for each source token
        rank = np.empty(TC, np.int64)
        rank[order] = np.arange(TC)
        csum = np.concatenate([[0], np.cumsum(cnt)])
        slot_of_src = offs[a] + (rank - csum[a])
        sT = np.zeros((CS, TDEV), np.float32)
        sT[:, slot_of_src] = np.maximum(s2[c], 0).T
        siT = np.zeros((CS, TDEV), np.float32)
        siT[:, slot_of_src] = np.maximum(si2[c], 0).T
        if use_f32r:
            sT = _tf32_round(sT)
            siT = _tf32_round(siT)
        rot9 = np.zeros((TDEV, 9), np.float32)
        rot9[slot_of_src] = rot2[c]
        tr3 = np.zeros((TDEV, 3), np.float32)
        tr3[slot_of_src] = tr2[c] * TRANS_SCALE
        dfl_tok = np.zeros((TDEV, 96), np.float32)
        dfl_tok[slot_of_src] = dfl[a]
        m = dict(shared)
        m.update(sT=sT, siT=siT, rot9=rot9, tr3=tr3, dfl_tok=dfl_tok)
        in_maps.append(m)
        slot_maps.append(slot_of_src)
    return in_maps, slot_maps, tuple(int(x) for x in caps), has_bias


def _frames7(rot_mats, trans):
    import jax
    import jax.numpy as jnp
    cpu = jax.devices("cpu")[0]
    with jax.default_device(cpu):
        rot = jnp.asarray(rot_mats)
        xx, xy, xz = rot[..., 0, 0], rot[..., 0, 1], rot[..., 0, 2]
        yx, yy, yz = rot[..., 1, 0], rot[..., 1, 1], rot[..., 1, 2]
        zx, zy, zz = rot[..., 2, 0], rot[..., 2, 1], rot[..., 2, 2]
        k = jnp.stack([
            jnp.stack([xx + yy + zz, zy - yz, xz - zx, yx - xy], -1),
            jnp.stack([zy - yz, xx - yy - zz, xy + yx, xz + zx], -1),
            jnp.stack([xz - zx, xy + yx, yy - xx - zz, yz + zy], -1),
            jnp.stack([yx - xy, xz + zx, yz + zy, zz - xx - yy], -1),
        ], -2) / 3.0
        _, vecs = jnp.linalg.eigh(k)
        quat = vecs[..., -1]
        bb_t = jnp.asarray(trans) * TRANS_SCALE
        out = jnp.concatenate([quat, bb_t], axis=-1)
        return np.asarray(out)


_CACHE = {}
TRACE = False
USE_F32R = False
LAST_RESULTS = None


def kernel(**inputs):
    global LAST_RESULTS
    in_maps, slot_maps, caps, has_bias = _stage(inputs, USE_F32R)
    key = (caps, has_bias, USE_F32R)
    if key not in _CACHE:
        _CACHE[key] = build_nc(list(caps), has_bias, USE_F32R)
    nc = _CACHE[key]
    rr = run_bass_kernel_spmd(nc, in_maps, core_ids=list(range(NCORE)),
                              trace=TRACE)
    LAST_RESULTS = rr
    res = rr.results

    sc44 = np.empty((B, N, 8, 4, 4), np.float32)
    unnorm = np.empty((B, N, 7, 2), np.float32)
    angles = np.empty((B, N, 7, 2), np.float32)
    pos = np.empty((B, N, NAT, 3), np.float32)
    for c in range(NCORE):
        sl = slot_maps[c]
        bs = slice(c * MC, (c + 1) * MC)
        sc44[bs] = res[c]["sc44_o"][sl].reshape(MC, N, 8, 4, 4)
        unnorm[bs] = res[c]["u_o"][sl].reshape(MC, N, 7, 2)
        angles[bs] = res[c]["ang_o"][sl].reshape(MC, N, 7, 2)
        pos[bs] = res[c]["pos_o"][:, sl].T.reshape(MC, N, NAT, 3)

    frames7 = _frames7(np.asarray(inputs["rot_mats"], np.float32),
                       np.asarray(inputs["trans"], np.float32))
    s_out = np.asarray(inputs["s"], np.float32)
    return frames7, sc44, unnorm, angles, pos, s_out
